# revision 2
# baseline (speedup 1.0000x reference)
"""Trainium2 Bass kernel for nn_Bilinear_54065048322517 (fp16 I/O, q=8 layout, v6).

Math:  out[b, j] = input2[b, j] * sum_{i,k} weights[i, j, k] * input1[b, i]
           =   input2 * (input1 @ weights.sum(axis=2))
Shapes: input1 (16384, 64), input2 (16384, 2048),
        weights (64, 2048, 64), out (16384, 2048); host API f32.

Sharding: split J=2048 into 8 shards of 256 (one per NeuronCore).
All device I/O in fp16 (host converts): per core input1 2MB + input2
shard 8MB + weights shard 2MB + out shard 8MB = 20MB HBM traffic
(vs 40MB f32).  Quantization rel-err ~5e-4, well under the 2e-2 gate.

Row-to-partition mapping uses q=8 consecutive DRAM rows per partition
(b = 1024n + 8p + q) so fp16 x2/out DMA descriptors stay 4KB runs
(8 rows x 512B) and x1 runs are 1KB; weights load in 4 chunks of 4KB
runs.  DMA descriptor cost ~= 10ns + bytes/(28.3GB/s) per engine, so
4KB runs keep the 16 DMA engines near their asymptotic rate.

Per-core kernel:
  phase A (interleaved with phase B group loads):
    - weights shard loaded as 4 chunks (128, 2048) fp16 on the ACT
      ring with partition 2i+h (h = j-half); DVE grouped-reduces
      over K -> w2tmp (128, 128) f32; two permutation-matrix matmuls
      P_h.T @ w2tmp + ACT copies de-interleave/duplicate into
      w2dup (128, 256) fp16 where partition r*64+i holds w2[i, :]
      for both r.
    - input1 loaded as 8 chunks (128, 1024) fp16 on the SP ring;
      64 TensorE (128,128) fp16 transposes -> x1T (128, 8192) fp16:
      x1T[(q&1)*64 + i, (4n + (q>>1))*128 + p] = x1[1024n + 8p + q, i].
  phase B (8 groups of 2 super-tiles = 2048 rows):
    - DMA x2 group -> xtile (128, 4096) fp16, SP ring (4KB runs)
    - per super-tile n, per half: 4 fp16 matmuls (K=64, M=128, N=256),
      concurrent pairs on DISJOINT PE row-groups into separate PSUM
      banks (col blocks ordered [q0 q2 q1 q3]):
        pt[:, blk(q')*256:+256] =
            x1T[(q&1)*64:+64, (4n+(q>>1))*128:+128].T @ w2dup[(q&1)*64:+64]
    - DVE: otile(fp16) = pt(f32, bank-deinterleaving AP) * xtile(fp16)
    - DMA otile -> out, one 0.5MB DMA per super-tile, ACT ring (4KB runs).
"""

import numpy as np

B, I, J, K = 16384, 64, 2048, 64
NCORES = 8
JS = J // NCORES          # 256 columns per core
Q = 8                     # consecutive DRAM rows per partition
SUP = 128 * Q             # 1024 rows per super-tile
NSUP = B // SUP           # 16 super-tiles
GROUP = 2                 # super-tiles per DMA group (2048 rows)
NG = NSUP // GROUP        # 8 groups
NWCHUNK = 8               # weights load chunks (all resident)
NXCHUNK = 8               # input1 load chunks
XBUFS = 5                 # xtile buffer depth
OBUFS = 3                 # otile buffer depth

_CACHE = {}


def _build_nc():
    from contextlib import ExitStack

    import concourse.mybir as mybir
    import concourse.tile as tile
    from concourse import bacc, masks

    f32 = mybir.dt.float32
    f16 = mybir.dt.float16
    nc = bacc.Bacc()

    x1 = nc.dram_tensor("input1", [B, I], f16, kind="ExternalInput")
    x2 = nc.dram_tensor("input2", [B, JS], f16, kind="ExternalInput")
    w = nc.dram_tensor("weights", [I, JS, K], f16, kind="ExternalInput")
    out = nc.dram_tensor("out", [B, JS], f16, kind="ExternalOutput")

    with tile.TileContext(nc) as tc, ExitStack() as ctx:
        const_pool = ctx.enter_context(tc.tile_pool(name="const", bufs=1))
        stage_pool = ctx.enter_context(tc.tile_pool(name="stage", bufs=1))
        wc_pool = ctx.enter_context(tc.tile_pool(name="wc", bufs=1))
        x_pool = ctx.enter_context(tc.tile_pool(name="xin", bufs=XBUFS))
        o_pool = ctx.enter_context(tc.tile_pool(name="oout", bufs=OBUFS))
        pc_pool = ctx.enter_context(tc.tile_pool(name="pc", bufs=3))
        ps_pool = ctx.enter_context(tc.tile_pool(name="ps", bufs=3, space="PSUM"))
        tr_pool = ctx.enter_context(tc.tile_pool(name="tr", bufs=2, space="PSUM"))

        identity = const_pool.tile([128, 128], f16)
        masks.make_identity(nc, identity[:])

        # permutation masks: P[h][2i+h, i] = P[h][2i+h, 64+i] = 1, else 0
        # (P_h.T @ w2tmp)[r*64+i, j''] = w2tmp[2i+h, j'']
        perm = []
        for h in range(2):
            ph = const_pool.tile([128, 128], f16, name=f"perm{h}")
            perm.append(ph)
            nc.gpsimd.memset(ph[:], 0.0)
            for r in range(2):
                # select p - 2*m - h == 0 over the (128, 64) column block
                nc.gpsimd.affine_select(
                    out=ph[:, r * 64 : (r + 1) * 64],
                    in_=ph[:, r * 64 : (r + 1) * 64],
                    compare_op=mybir.AluOpType.not_equal,
                    fill=1.0,
                    base=-h,
                    pattern=[[-2, 64]],
                    channel_multiplier=1,
                )

        # ---- input1 chunk loads (SP ring) + x2 prefetch interleave ----
        x1stage = stage_pool.tile([128, B * I // 128], f16)  # (128, 8192)
        x1_r = x1.rearrange("(n p q) i -> p n q i", p=128, q=Q)  # (128,16,8,64)
        xcsz = B * I // 128 // NXCHUNK  # 1024 elems/partition per chunk
        xnsz = NSUP // NXCHUNK          # 2 super-tiles per chunk

        def load_x1_chunk(k):
            nc.scalar.dma_start(
                out=x1stage[:, k * xcsz : (k + 1) * xcsz].rearrange(
                    "p (n q i) -> p n q i", q=Q, i=I
                ),
                in_=x1_r[:, k * xnsz : (k + 1) * xnsz],
            )

        x2_r = x2.rearrange(
            "(g s p q) j -> g p s q j", g=NG, s=GROUP, p=128, q=Q
        )
        out_r = out.rearrange(
            "(g sh p q) j -> g sh p q j", g=NG, sh=GROUP, p=128, q=Q
        )

        xtiles = []

        def load(g):
            assert len(xtiles) == g
            xt = x_pool.tile([128, GROUP * Q * JS], f16, name=f"xt{g}", tag="xt")
            xtiles.append(xt)
            nc.sync.dma_start(
                out=xt[:].rearrange("p (s q j) -> p s q j", s=GROUP, q=Q),
                in_=x2_r[g],
            )

        # ---- weights load (ACT ring, 128 partitions) + K-reduction ----
        # chunk c: partition 2i+h <- W[i, 128h + c*32 + j'', :], 4KB runs.
        # w chunks lead the ACT queue (w2dup is the prologue critical path),
        # then x1 chunks; the SP queue starts on x2 groups immediately.
        w_v = w.rearrange("i (h c j) k -> c i h (j k)", h=2, c=NWCHUNK)
        w2tmp = const_pool.tile([128, JS // 2], f16)  # (128, 128), part 2i+h
        csz = JS * K // NWCHUNK // 2  # 2048 elems per partition per chunk
        jcs = JS // NWCHUNK // 2      # 32 w2tmp columns per chunk
        wchunks = []
        for c in range(NWCHUNK):
            wchunk = wc_pool.tile(
                [128, csz], f16, name=f"wchunk{c}", tag=f"wchunk{c}"
            )
            wchunks.append(wchunk)
            nc.scalar.dma_start(
                out=wchunk[:],
                in_=w_v[c].rearrange("i h f -> (i h) f"),
            )

        load(0)
        for k in range(NXCHUNK):
            load_x1_chunk(k)
        for g0 in range(1, XBUFS):
            load(g0)

        # one 2x_1p tensor_add fold (k 64->32) then a half-size reduce
        # (tensor_reduce has no DVE fast modes) -- ~30% less prologue DVE
        kh_pool = ctx.enter_context(tc.tile_pool(name="kh", bufs=2))
        with nc.allow_low_precision(reason="fp16 K-reduce; rel-err gate is 2e-2"):
            for c in range(NWCHUNK):
                wv = wchunks[c][:].rearrange("p (j k) -> p j k", k=K)
                khalf = kh_pool.tile([128, csz // 2], f16, name="khalf", tag="kh")
                kv = khalf[:].rearrange("p (j k) -> p j k", k=K // 2)
                nc.vector.tensor_add(kv, wv[:, :, 0 : K // 2], wv[:, :, K // 2 : K])
                nc.vector.tensor_reduce(
                    out=w2tmp[:, c * jcs : (c + 1) * jcs],
                    in_=kv,
                    axis=mybir.AxisListType.X,
                    op=mybir.AluOpType.add,
                )

        # de-interleave + duplicate via permutation matmuls:
        # w2dup[r*64+i, 128h+j''] = w2tmp[2i+h, j'']
        w2dup = const_pool.tile([128, JS], f16)
        for h in range(2):
            pw = tr_pool.tile([128, 512], f32, name="tt", tag="tt")
            nc.tensor.matmul(
                pw[:, 0:128], lhsT=perm[h][:], rhs=w2tmp[:],
                start=True, stop=True,
            )
            nc.scalar.copy(w2dup[:, h * 128 : (h + 1) * 128], pw[:, 0:128])

        # ---- transposes + groups, interleaved ----
        # x1T[(q&1)*64 + i, m*128 + p] = x1[1024*(m>>2) + 8p + 2*(m&3) + (q&1), i]
        x1T = const_pool.tile([128, 64 * 128], f16)  # (128, 8192)

        def transpose_batch(m):
            tt = tr_pool.tile([128, 512], f16, name="tt", tag="tt")
            for s in range(4):
                blk = m * 4 + s
                nc.tensor.transpose(
                    tt[:, s * 128 : (s + 1) * 128],
                    x1stage[:, blk * 128 : (blk + 1) * 128],
                    identity[:],
                )
            nc.scalar.copy(x1T[:, m * 512 : (m + 1) * 512], tt[:])

        # psum column block for q' (within a half): concurrent pair
        # (q'=0, q'=1) goes to banks 0/1; (q'=2, q'=3) to banks 0/1.
        PSBLK = [0, 2, 1, 3]

        def process(g):
            xtile = xtiles[g]
            ot = o_pool.tile([128, GROUP * Q * JS], f16, name=f"ot{g}", tag="ot")
            if g + XBUFS < NG:
                load(g + XBUFS)
            for t in range(2 * GROUP):
                n = GROUP * g + (t >> 1)
                half = t & 1
                pt = ps_pool.tile([128, 4 * JS], f32)  # (128, 1024), 2 banks
                for qp in range(4):
                    q = half * 4 + qp
                    nc.tensor.matmul(
                        pt[:, PSBLK[qp] * JS : (PSBLK[qp] + 1) * JS],
                        lhsT=x1T[
                            (q & 1) * 64 : (q & 1) * 64 + 64,
                            (4 * n + (q >> 1)) * 128 : (4 * n + (q >> 1) + 1) * 128,
                        ],
                        rhs=w2dup[(q & 1) * 64 : (q & 1) * 64 + 64, :],
                        start=True,
                        stop=True,
                    )
                # pt col blocks hold [q'0, q'2, q'1, q'3]; the (u v) -> (v u)
                # swap iterates them back in natural q' order.  Most tiles:
                # ACT de-interleaves PSUM -> SBUF f16, then DVE multiplies
                # all-f16 all-SBUF packed (2x_1p perf mode).  One tile per
                # group: DVE reads PSUM directly to balance ACT vs DVE.
                if t != 1:
                    ptc = pc_pool.tile([128, 1024], f16, name="ptc", tag="ptc")
                    nc.scalar.copy(
                        ptc[:].rearrange("p (a b j) -> p a b j", a=2, b=2),
                        pt[:].rearrange("p (u v j) -> p v u j", u=2, v=2),
                    )
                    nc.vector.tensor_mul(
                        ot[:, t * 1024 : (t + 1) * 1024],
                        ptc[:],
                        xtile[:, t * 1024 : (t + 1) * 1024],
                    )
                else:
                    nc.vector.tensor_mul(
                        ot[:, t * 1024 : (t + 1) * 1024].rearrange(
                            "p (a b j) -> p a b j", a=2, b=2
                        ),
                        pt[:].rearrange("p (u v j) -> p v u j", u=2, v=2),
                        xtile[:, t * 1024 : (t + 1) * 1024].rearrange(
                            "p (a b j) -> p a b j", a=2, b=2
                        ),
                    )
                if half == 1:
                    # store triggers ride the sync engine: its only other
                    # work is x2 load triggers, so an embedded DVE wait here
                    # never blocks the ACT copy stream.
                    sh = t >> 1
                    nc.sync.dma_start(
                        out=out_r[g, sh],
                        in_=ot[:, sh * Q * JS : (sh + 1) * Q * JS].rearrange(
                            "p (q j) -> p q j", q=Q
                        ),
                    )

        for g in range(NG):
            transpose_batch(2 * g)
            transpose_batch(2 * g + 1)
            if g >= 1:
                process(g - 1)
        process(NG - 1)

    nc.compile()
    return nc


def _get_nc():
    if "nc" not in _CACHE:
        _CACHE["nc"] = _build_nc()
    return _CACHE["nc"]


def _make_in_maps(input1, input2, weights):
    input1 = np.ascontiguousarray(input1).astype(np.float16)
    in_maps = []
    for c in range(NCORES):
        sl = slice(c * JS, (c + 1) * JS)
        in_maps.append(
            {
                "input1": input1,
                "input2": input2[:, sl].astype(np.float16),
                "weights": weights[:, sl, :].astype(np.float16),
            }
        )
    return in_maps


def run(input1, input2, weights, trace=False, **spmd_kwargs):
    from concourse.bass_utils import run_bass_kernel_spmd

    nc = _get_nc()
    in_maps = _make_in_maps(input1, input2, weights)
    res = run_bass_kernel_spmd(
        nc, in_maps, core_ids=list(range(NCORES)), trace=trace, **spmd_kwargs
    )
    outs = [res.results[c]["out"] for c in range(NCORES)]
    full = np.concatenate(outs, axis=1).astype(np.float32)
    return full, res


def kernel(input1, input2, weights):
    full, _ = run(input1, input2, weights, trace=False)
    return full


# revision 3
# speedup vs baseline: 1.0382x; 1.0382x over previous
"""Trainium2 Bass kernel for nn_Bilinear_54065048322517 (fp16 I/O, q=8 layout, v8).

Math:  out[b, j] = input2[b, j] * sum_{i,k} weights[i, j, k] * input1[b, i]
           =   input2 * (input1 @ weights.sum(axis=2))
Shapes: input1 (16384, 64), input2 (16384, 2048),
        weights (64, 2048, 64), out (16384, 2048); host API f32.

Sharding: split J=2048 into 8 shards of 256 (one per NeuronCore).
All device I/O in fp16 (host converts): per core input1 2MB + input2
shard 8MB + weights shard 2MB + out shard 8MB = 20MB HBM traffic
(vs 40MB f32).  Quantization rel-err ~5e-4, well under the 2e-2 gate.

Row-to-partition mapping uses q=8 consecutive DRAM rows per partition
(b = 1024n + 8p + q) so fp16 x2/out DMA descriptors stay 4KB runs
(8 rows x 512B) and x1 runs are 1KB; weights load in 4 chunks of 4KB
runs.  DMA descriptor cost ~= 10ns + bytes/(28.3GB/s) per engine, so
4KB runs keep the 16 DMA engines near their asymptotic rate.

Per-core kernel:
  phase A (interleaved with phase B group loads):
    - weights shard loaded as 4 chunks (128, 2048) fp16 on the ACT
      ring with partition 2i+h (h = j-half); DVE grouped-reduces
      over K -> w2tmp (128, 128) f32; two permutation-matrix matmuls
      P_h.T @ w2tmp + ACT copies de-interleave/duplicate into
      w2dup (128, 256) fp16 where partition r*64+i holds w2[i, :]
      for both r.
    - input1 loaded as 8 chunks (128, 1024) fp16 on the SP ring;
      64 TensorE (128,128) fp16 transposes -> x1T (128, 8192) fp16:
      x1T[(q&1)*64 + i, (4n + (q>>1))*128 + p] = x1[1024n + 8p + q, i].
  phase B (8 groups of 2 super-tiles = 2048 rows):
    - DMA x2 group -> xtile (128, 4096) fp16, SP ring (4KB runs)
    - per super-tile n, per half: 4 fp16 matmuls (K=64, M=128, N=256),
      concurrent pairs on DISJOINT PE row-groups into separate PSUM
      banks (col blocks ordered [q0 q2 q1 q3]):
        pt[:, blk(q')*256:+256] =
            x1T[(q&1)*64:+64, (4n+(q>>1))*128:+128].T @ w2dup[(q&1)*64:+64]
    - DVE: otile(fp16) = pt(f32, bank-deinterleaving AP) * xtile(fp16)
    - DMA otile -> out, one 0.5MB DMA per super-tile, ACT ring (4KB runs).
"""

import numpy as np

B, I, J, K = 16384, 64, 2048, 64
NCORES = 8
JS = J // NCORES          # 256 columns per core
Q = 8                     # consecutive DRAM rows per partition
SUP = 128 * Q             # 1024 rows per super-tile
NSUP = B // SUP           # 16 super-tiles
GROUP = 2                 # super-tiles per DMA group (2048 rows)
NG = NSUP // GROUP        # 8 groups
NWCHUNK = 4               # weights load chunks (all resident)
NXCHUNK = 8               # input1 load chunks
XBUFS = 5                 # xtile buffer depth
OBUFS = 3                 # otile buffer depth

_CACHE = {}


def _build_nc():
    from contextlib import ExitStack

    import concourse.mybir as mybir
    import concourse.tile as tile
    from concourse import bacc, masks

    f32 = mybir.dt.float32
    f16 = mybir.dt.float16
    nc = bacc.Bacc()

    x1 = nc.dram_tensor("input1", [B, I], f16, kind="ExternalInput")
    x2 = nc.dram_tensor("input2", [B, JS], f16, kind="ExternalInput")
    w = nc.dram_tensor("weights", [I, JS, K], f16, kind="ExternalInput")
    out = nc.dram_tensor("out", [B, JS], f16, kind="ExternalOutput")

    with tile.TileContext(nc) as tc, ExitStack() as ctx:
        const_pool = ctx.enter_context(tc.tile_pool(name="const", bufs=1))
        stage_pool = ctx.enter_context(tc.tile_pool(name="stage", bufs=1))
        wc_pool = ctx.enter_context(tc.tile_pool(name="wc", bufs=1))
        x_pool = ctx.enter_context(tc.tile_pool(name="xin", bufs=XBUFS))
        o_pool = ctx.enter_context(tc.tile_pool(name="oout", bufs=OBUFS))
        pc_pool = ctx.enter_context(tc.tile_pool(name="pc", bufs=3))
        ps_pool = ctx.enter_context(tc.tile_pool(name="ps", bufs=3, space="PSUM"))
        tr_pool = ctx.enter_context(tc.tile_pool(name="tr", bufs=2, space="PSUM"))

        identity = const_pool.tile([128, 128], f16)
        masks.make_identity(nc, identity[:])

        # permutation masks: P[h][2i+h, i] = P[h][2i+h, 64+i] = 1, else 0
        # (P_h.T @ w2tmp)[r*64+i, j''] = w2tmp[2i+h, j'']
        perm = []
        for h in range(2):
            ph = const_pool.tile([128, 128], f16, name=f"perm{h}")
            perm.append(ph)
            nc.gpsimd.memset(ph[:], 0.0)
            for r in range(2):
                # select p - 2*m - h == 0 over the (128, 64) column block
                nc.gpsimd.affine_select(
                    out=ph[:, r * 64 : (r + 1) * 64],
                    in_=ph[:, r * 64 : (r + 1) * 64],
                    compare_op=mybir.AluOpType.not_equal,
                    fill=1.0,
                    base=-h,
                    pattern=[[-2, 64]],
                    channel_multiplier=1,
                )

        # ---- input1 chunk loads (SP ring) + x2 prefetch interleave ----
        x1stage = stage_pool.tile([128, B * I // 128], f16)  # (128, 8192)
        x1_r = x1.rearrange("(n p q) i -> p n q i", p=128, q=Q)  # (128,16,8,64)
        xcsz = B * I // 128 // NXCHUNK  # 1024 elems/partition per chunk
        xnsz = NSUP // NXCHUNK          # 2 super-tiles per chunk

        def load_x1_chunk(k):
            nc.scalar.dma_start(
                out=x1stage[:, k * xcsz : (k + 1) * xcsz].rearrange(
                    "p (n q i) -> p n q i", q=Q, i=I
                ),
                in_=x1_r[:, k * xnsz : (k + 1) * xnsz],
            )

        x2_r = x2.rearrange(
            "(g s p q) j -> g p s q j", g=NG, s=GROUP, p=128, q=Q
        )
        out_r = out.rearrange(
            "(g sh p q) j -> g sh p q j", g=NG, sh=GROUP, p=128, q=Q
        )

        xtiles = []

        def load(g):
            assert len(xtiles) == g
            xt = x_pool.tile([128, GROUP * Q * JS], f16, name=f"xt{g}", tag="xt")
            xtiles.append(xt)
            nc.sync.dma_start(
                out=xt[:].rearrange("p (s q j) -> p s q j", s=GROUP, q=Q),
                in_=x2_r[g],
            )

        # ---- weights load (ACT ring, 128 partitions) + K-reduction ----
        # chunk c: partition 2i+h <- W[i, 128h + c*32 + j'', :], 4KB runs.
        # w chunks lead the ACT queue (w2dup is the prologue critical path),
        # then x1 chunks; the SP queue starts on x2 groups immediately.
        w_v = w.rearrange("i (h c j) k -> c i h (j k)", h=2, c=NWCHUNK)
        w2tmp = const_pool.tile([128, JS // 2], f16)  # (128, 128), part 2i+h
        csz = JS * K // NWCHUNK // 2  # 2048 elems per partition per chunk
        jcs = JS // NWCHUNK // 2      # 32 w2tmp columns per chunk
        wchunks = []
        for c in range(NWCHUNK):
            wchunk = wc_pool.tile(
                [128, csz], f16, name=f"wchunk{c}", tag=f"wchunk{c}"
            )
            wchunks.append(wchunk)
            nc.scalar.dma_start(
                out=wchunk[:],
                in_=w_v[c].rearrange("i h f -> (i h) f"),
            )

        load(0)
        for k in range(NXCHUNK):
            load_x1_chunk(k)
        for g0 in range(1, XBUFS):
            load(g0)

        # two 2x_1p tensor_add folds (k 64->32->16) then a quarter-size
        # reduce (tensor_reduce has no DVE fast modes); fewer, bigger
        # chunks cut instruction+semaphore overhead on the critical path
        kh_pool = ctx.enter_context(tc.tile_pool(name="kh", bufs=2))
        kq_pool = ctx.enter_context(tc.tile_pool(name="kq", bufs=2))
        with nc.allow_low_precision(reason="fp16 K-reduce; rel-err gate is 2e-2"):
            for c in range(NWCHUNK):
                wv = wchunks[c][:].rearrange("p (j k) -> p j k", k=K)
                khalf = kh_pool.tile([128, csz // 2], f16, name="khalf", tag="kh")
                kv = khalf[:].rearrange("p (j k) -> p j k", k=K // 2)
                nc.vector.tensor_add(kv, wv[:, :, 0 : K // 2], wv[:, :, K // 2 : K])
                kquar = kq_pool.tile([128, csz // 4], f16, name="kquar", tag="kq")
                qv = kquar[:].rearrange("p (j k) -> p j k", k=K // 4)
                nc.vector.tensor_add(qv, kv[:, :, 0 : K // 4], kv[:, :, K // 4 : K // 2])
                nc.vector.tensor_reduce(
                    out=w2tmp[:, c * jcs : (c + 1) * jcs],
                    in_=qv,
                    axis=mybir.AxisListType.X,
                    op=mybir.AluOpType.add,
                )

        # de-interleave + duplicate via permutation matmuls:
        # w2dup[r*64+i, 128h+j''] = w2tmp[2i+h, j'']
        w2dup = const_pool.tile([128, JS], f16)
        for h in range(2):
            pw = tr_pool.tile([128, 512], f32, name="tt", tag="tt")
            nc.tensor.matmul(
                pw[:, 0:128], lhsT=perm[h][:], rhs=w2tmp[:],
                start=True, stop=True,
            )
            nc.scalar.copy(w2dup[:, h * 128 : (h + 1) * 128], pw[:, 0:128])

        # ---- transposes + groups, interleaved ----
        # x1T[(q&1)*64 + i, m*128 + p] = x1[1024*(m>>2) + 8p + 2*(m&3) + (q&1), i]
        x1T = const_pool.tile([128, 64 * 128], f16)  # (128, 8192)

        def transpose_batch(m):
            tt = tr_pool.tile([128, 512], f16, name="tt", tag="tt")
            for s in range(4):
                blk = m * 4 + s
                nc.tensor.transpose(
                    tt[:, s * 128 : (s + 1) * 128],
                    x1stage[:, blk * 128 : (blk + 1) * 128],
                    identity[:],
                )
            nc.scalar.copy(x1T[:, m * 512 : (m + 1) * 512], tt[:])

        # psum column block for q' (within a half): concurrent pair
        # (q'=0, q'=1) goes to banks 0/1; (q'=2, q'=3) to banks 0/1.
        PSBLK = [0, 2, 1, 3]

        def process(g):
            xtile = xtiles[g]
            ot = o_pool.tile([128, GROUP * Q * JS], f16, name=f"ot{g}", tag="ot")
            if g + XBUFS < NG:
                load(g + XBUFS)
            for t in range(2 * GROUP):
                n = GROUP * g + (t >> 1)
                half = t & 1
                pt = ps_pool.tile([128, 4 * JS], f32)  # (128, 1024), 2 banks
                for qp in range(4):
                    q = half * 4 + qp
                    nc.tensor.matmul(
                        pt[:, PSBLK[qp] * JS : (PSBLK[qp] + 1) * JS],
                        lhsT=x1T[
                            (q & 1) * 64 : (q & 1) * 64 + 64,
                            (4 * n + (q >> 1)) * 128 : (4 * n + (q >> 1) + 1) * 128,
                        ],
                        rhs=w2dup[(q & 1) * 64 : (q & 1) * 64 + 64, :],
                        start=True,
                        stop=True,
                    )
                # pt col blocks hold [q'0, q'2, q'1, q'3]; the (u v) -> (v u)
                # swap iterates them back in natural q' order.  Most tiles:
                # ACT de-interleaves PSUM -> SBUF f16, then DVE multiplies
                # all-f16 all-SBUF packed (2x_1p perf mode).  One tile per
                # group: DVE reads PSUM directly to balance ACT vs DVE.
                if t != 1:
                    ptc = pc_pool.tile([128, 1024], f16, name="ptc", tag="ptc")
                    nc.scalar.copy(
                        ptc[:].rearrange("p (a b j) -> p a b j", a=2, b=2),
                        pt[:].rearrange("p (u v j) -> p v u j", u=2, v=2),
                    )
                    nc.vector.tensor_mul(
                        ot[:, t * 1024 : (t + 1) * 1024],
                        ptc[:],
                        xtile[:, t * 1024 : (t + 1) * 1024],
                    )
                else:
                    nc.vector.tensor_mul(
                        ot[:, t * 1024 : (t + 1) * 1024].rearrange(
                            "p (a b j) -> p a b j", a=2, b=2
                        ),
                        pt[:].rearrange("p (u v j) -> p v u j", u=2, v=2),
                        xtile[:, t * 1024 : (t + 1) * 1024].rearrange(
                            "p (a b j) -> p a b j", a=2, b=2
                        ),
                    )
                if half == 1:
                    # store triggers ride the sync engine: its only other
                    # work is x2 load triggers, so an embedded DVE wait here
                    # never blocks the ACT copy stream.
                    sh = t >> 1
                    nc.sync.dma_start(
                        out=out_r[g, sh],
                        in_=ot[:, sh * Q * JS : (sh + 1) * Q * JS].rearrange(
                            "p (q j) -> p q j", q=Q
                        ),
                    )

        for g in range(NG):
            transpose_batch(2 * g)
            transpose_batch(2 * g + 1)
            if g >= 1:
                process(g - 1)
        process(NG - 1)

    nc.compile()
    return nc


def _get_nc():
    if "nc" not in _CACHE:
        _CACHE["nc"] = _build_nc()
    return _CACHE["nc"]


def _make_in_maps(input1, input2, weights):
    input1 = np.ascontiguousarray(input1).astype(np.float16)
    in_maps = []
    for c in range(NCORES):
        sl = slice(c * JS, (c + 1) * JS)
        in_maps.append(
            {
                "input1": input1,
                "input2": input2[:, sl].astype(np.float16),
                "weights": weights[:, sl, :].astype(np.float16),
            }
        )
    return in_maps


def run(input1, input2, weights, trace=False, **spmd_kwargs):
    from concourse.bass_utils import run_bass_kernel_spmd

    nc = _get_nc()
    in_maps = _make_in_maps(input1, input2, weights)
    res = run_bass_kernel_spmd(
        nc, in_maps, core_ids=list(range(NCORES)), trace=trace, **spmd_kwargs
    )
    outs = [res.results[c]["out"] for c in range(NCORES)]
    full = np.concatenate(outs, axis=1).astype(np.float32)
    return full, res


def kernel(input1, input2, weights):
    full, _ = run(input1, input2, weights, trace=False)
    return full
